# revision 49
# baseline (speedup 1.0000x reference)
"""Trainium2 Bass kernel for nn_DAWNLayer (moe_routing).

Sharding: data-parallel over (batch, sequence) across 8 cores — core c
handles batch c//4, query rows (c%4)*512..+512. K/V are computed
redundantly per core for the core's batch (full S=2048). All weights
replicated. Inputs are pre-transposed / pre-tiled on the host so every
DMA is a contiguous per-partition stream; each core's xT is cyclically
rolled so its own query block is always cols 0:512.

Device algorithm (activations transposed [feature, row]):
  Shared LN stats (LN1/LN2 differ only in affine, folded into weights /
  FFN-input eviction). Attention scores computed transposed st[t,q],
  softmax without max-subtraction, denominator via a ones-column folded
  into V_ext (65-wide head blocks), division folded into ctx eviction.
  Scores for head pairs are issued adjacently: K=64 stationary tiles at
  base partitions 0/64 auto-derive disjoint PE row strips and run
  concurrently. exp() is done on [128,1024] head-pair tiles to amortize
  the ACT fixed cost. K/Q projection matmul groups are interleaved into
  the attention instruction stream so the PE never idles while ACT
  catches up on exp (keeps the HAM clock gate warm).
  Router: dense top-8 masked softmax via vector.max + match_replace,
  then info = emb.T @ w_dense as a dense matmul (no gather).
  All matmuls bf16 (fp32 PSUM accumulation); LN stats in fp32.
"""

import os

os.environ.setdefault("MYCRO_LOCAL_CACHE", "1")

import numpy as np
import ml_dtypes

import concourse.bass as bass
import concourse.mybir as mybir
import concourse.tile as tile
from concourse import bacc
from concourse.bass_utils import run_bass_kernel_spmd
from concourse.masks import make_identity

dt = mybir.dt
BF = ml_dtypes.bfloat16

B, S, D = 2, 2048, 1024
DFF = 4096
H, DH = 16, 64
NN, K = 256, 8
EPS = 1e-5
QB = 512           # own query rows per core
NDT = D // 128     # 8 d-tiles
NTT = S // 128     # 16 t row-tiles
NFT = DFF // 128   # 32 dff tiles
NQS = QB // 128    # 4 q subtiles
VW = 65 * H        # 1040 V_ext width
RW = S - QB        # 1536 non-own rows

F32, F32R, BF16 = dt.float32, dt.float32r, dt.bfloat16


def r(ap):
    return ap.bitcast(F32R)


def build_program():
    nc = bacc.Bacc("TRN2", target_bir_lowering=False, debug=False, num_devices=8)

    def din(name, shape, dtype):
        return nc.dram_tensor(name, list(shape), dtype, kind="ExternalInput").ap()

    t = {}
    # pre-tiled inputs: row index = out-tile*128 + partition, cols contiguous
    t["x_all"] = din("x_all", (128, NDT * S), F32R)
    t["xT_own"] = din("xT_own", (128, NDT * QB), F32R)
    t["wqT"] = din("wqT", (NDT * 128, NDT * 128), BF16)
    t["wkT"] = din("wkT", (NDT * 128, NDT * 128), BF16)
    t["wvT_ext"] = din("wvT_ext", (128, NDT * VW), BF16)
    t["bv_ext"] = din("bv_ext", (1, VW), BF16)
    t["wse1"] = din("wse1", (128, NDT * NN), BF16)
    t["wse2"] = din("wse2", (128, NDT * NN), BF16)
    t["bse"] = din("bse", (1, NN), BF16)
    t["embWnp"] = din("embWnp", (NN, D), BF16)
    t["wupT"] = din("wupT", (NFT * 128, NDT * 128), BF16)
    t["wdownT"] = din("wdownT", (DFF, D), BF16)
    for nm in ("bq_c", "bk_c", "b2np_c", "bdown_c", "g2_c"):
        t[nm] = din(nm, (128, NDT), F32)
    t["bup_c"] = din("bup_c", (128, NFT), F32)
    t["ones_col"] = din("ones_col", (128, 1), F32R)
    t["yT"] = nc.dram_tensor("yT", [D, QB], F32, kind="ExternalOutput").ap()
    if os.environ.get("DAWN_DEBUG"):
        for nm, shape in (("dbg_Kt", (128, NDT * S)),
                          ("dbg_Qt", (128, NDT * QB)),
                          ("dbg_ctxT", (128, NDT * QB)),
                          ("dbg_xh", (128, NDT * QB)),
                          ("dbg_wbf", (128, NQS * NN)),
                          ("dbg_wT", (128, 2 * QB)),
                          ("dbg_xaugT", (128, NDT * QB)),
                          ("dbg_hT", (128, NFT * QB))):
            t[nm] = nc.dram_tensor(nm, list(shape), BF16,
                                   kind="ExternalOutput").ap()

    with tile.TileContext(nc) as tc:
        emit(tc, t)
    nc.compile()
    return nc


def emit(tc, t):
    from contextlib import ExitStack
    nc = tc.nc
    A = mybir.AluOpType
    AF = mybir.ActivationFunctionType
    X = mybir.AxisListType.X

    est = ExitStack()
    # ---- whole-kernel pools (left side) ----
    cp = est.enter_context(tc.tile_pool(name="consts", bufs=1))

    # x region DMAs go FIRST on the sync queue (every dma_start costs ~1us
    # of issue time on its engine queue); consts ride the scalar queue.
    ones_col = cp.tile([128, 1], F32R, tag="ones_col")
    ones_row = cp.tile([1, 128], BF16, tag="ones_row")
    eps_t = cp.tile([1, 1], F32, tag="eps_t")
    nc.vector.memset(eps_t[:], EPS)
    nc.gpsimd.dma_start(ones_col[:], t["ones_col"])
    nc.vector.memset(ones_row[:], 1.0)

    cols = {}
    for nm in ("bq_c", "bk_c", "b2np_c", "bdown_c", "g2_c"):
        cols[nm] = cp.tile([128, NDT], F32, tag=nm, name=nm)
        nc.gpsimd.dma_start(cols[nm][:], t[nm])
    bup_sb = cp.tile([128, NFT], F32, tag="bup")
    nc.gpsimd.dma_start(bup_sb[:], t["bup_c"])
    bv_sb = cp.tile([1, VW], BF16, tag="bv")
    nc.gpsimd.dma_start(bv_sb[:], t["bv_ext"])

    # ---- right-side: bf16 xhat-own (P1 -> router) ----
    rstk = ExitStack()
    rp1 = rstk.enter_context(tc.tile_pool(name="xhbf", bufs=1, side="right"))
    xho_bf = rp1.tile([128, NDT * QB], BF16, tag="xho_bf")

    def xhsl(i, c):  # xhat bf16 [d-tile i, t-chunk c of 512]
        if c == 0:
            return xho_bf[:, QB * i:QB * i + 512]
        return xhr_bf[:, RW * i + 512 * (c - 1):RW * i + 512 * c]

    def xhrow(i, tt):  # xhat bf16 [d-tile i, 128 rows of row-tile tt]
        if tt < 4:
            return xho_bf[:, QB * i + 128 * tt:QB * i + 128 * (tt + 1)]
        return xhr_bf[:, RW * i + 128 * (tt - 4):RW * i + 128 * (tt - 3)]

    # ============ P1+V: shared LN stats, xhat, V (chunk-pipelined) ========
    attn_stack = ExitStack()
    vxp = attn_stack.enter_context(tc.tile_pool(name="vxp", bufs=1))
    Vx = vxp.tile([128, NTT * VW], BF16, tag="Vx")
    # xhat-rest lives on the left stack: freed with the attention pools
    xhr_bf = vxp.tile([128, NDT * RW], BF16, tag="xhr_bf")

    with tc.tile_pool(name="p1sb", bufs=1) as p1, \
         tc.tile_pool(name="rows", bufs=12) as rows, \
         tc.tile_pool(name="bcp", bufs=2) as bcp, \
         tc.tile_pool(name="xsc", bufs=2) as xsc:
        x_all = p1.tile([128, NDT * S], F32R, tag="x_all")
        # per-d-tile region DMAs so LN stats/squares chase the transfer
        for i in range(NDT):
            nc.sync.dma_start(x_all[:, S * i:S * (i + 1)],
                              t["x_all"][:, S * i:S * (i + 1)])
        wv_all = p1.tile([128, NDT * VW], BF16, tag="wv_all")
        nc.gpsimd.dma_start(wv_all[:], t["wvT_ext"])

        def xsl(i, c):
            return x_all[:, S * i + 512 * c:S * i + 512 * (c + 1)]

        m_cs, v_cs = [], []
        with tc.tile_pool(name="xsqp", bufs=1) as xsqp, \
             tc.tile_pool(name="p1ps", bufs=4, space="PSUM") as ps1:
            sx_ps = [ps1.tile([1, 512], F32, tag="sx", name="sx")
                     for _ in range(4)]
            sxx_ps = [ps1.tile([1, 512], F32, tag="sxx", name="sxx")
                      for _ in range(4)]
            for i in range(NDT):
                xsq = xsqp.tile([128, S], F32R, tag="xsq")
                nc.scalar.activation(xsq[:], x_all[:, S * i:S * (i + 1)],
                                     AF.Square)
                for c in range(4):
                    nc.tensor.matmul(sx_ps[c][:], r(ones_col[:]), r(xsl(i, c)),
                                     start=(i == 0), stop=(i == NDT - 1))
                    nc.tensor.matmul(sxx_ps[c][:], r(ones_col[:]),
                                     r(xsq[:, 512 * c:512 * (c + 1)]),
                                     start=(i == 0), stop=(i == NDT - 1))
            for c in range(4):
                m_c = rows.tile([1, 512], F32, tag="rows", name="m_c")
                v_c = rows.tile([1, 512], F32, tag="rows", name="v_c")
                nc.scalar.activation(m_c[:], sx_ps[c][:], AF.Copy, scale=1.0 / D)
                nc.scalar.activation(v_c[:], sxx_ps[c][:], AF.Copy, scale=1.0 / D)
                m_cs.append(m_c)
                v_cs.append(v_c)

        with tc.tile_pool(name="vps", bufs=2, space="PSUM") as vps:
            for c in range(4):
                m_c, v_c = m_cs[c], v_cs[c]
                msq = rows.tile([1, 512], F32, tag="rows", name="msq")
                nc.vector.tensor_mul(msq[:], m_c[:], m_c[:])
                nc.vector.tensor_sub(v_c[:], v_c[:], msq[:])
                sdev = rows.tile([1, 512], F32, tag="rows", name="sdev")
                nc.scalar.activation(sdev[:], v_c[:], AF.Sqrt, bias=eps_t[:])
                rstd_c = rows.tile([1, 512], F32, tag="rows", name="rstd_c")
                nc.vector.reciprocal(rstd_c[:], sdev[:])
                mhat_c = rows.tile([1, 512], F32, tag="rows", name="mhat_c")
                nc.vector.tensor_mul(mhat_c[:], m_c[:], rstd_c[:])
                rstd_bc = bcp.tile([128, 512], F32, tag="rstd_bc", name="rstd_bc")
                mhat_bc = bcp.tile([128, 512], F32, tag="mhat_bc", name="mhat_bc")
                nc.gpsimd.partition_broadcast(rstd_bc[:], rstd_c[:])
                nc.gpsimd.partition_broadcast(mhat_bc[:], mhat_c[:])
                for i in range(NDT):
                    o = xsc.tile([128, 512], F32, tag="xsc", name="o")
                    nc.vector.tensor_mul(o[:], xsl(i, c), rstd_bc[:])
                    nc.vector.tensor_sub(xhsl(i, c), o[:], mhat_bc[:])
                # V for this chunk's 4 row-tiles (needs only chunk-c xhat)
                for tt in range(4 * c, 4 * (c + 1)):
                    for half in range(2):
                        ps = vps.tile([128, 1024], F32, tag="v", name="psv")
                        for cc in range(2):
                            c4 = 2 * half + cc
                            dst = ps[:, 512 * cc:512 * cc + 260]
                            for i in range(NDT):
                                nc.tensor.matmul(
                                    dst, xhrow(i, tt),
                                    wv_all[:, VW * i + 260 * c4:VW * i + 260 * (c4 + 1)],
                                    start=(i == 0), stop=False)
                            nc.tensor.matmul(dst, ones_row[:],
                                             bv_sb[:, 260 * c4:260 * (c4 + 1)],
                                             start=False, stop=True)
                        src2 = ps[:].rearrange("p (c n) -> p c n", c=2)[:, :, 0:260]
                        dst2 = Vx[:, VW * tt + 520 * half:VW * tt + 520 * (half + 1)]
                        nc.scalar.activation(
                            dst2.rearrange("p (c n) -> p c n", c=2), src2, AF.Copy)

    # ============ P2+P3: Q/K projections interleaved with attention ========
    c2 = attn_stack.enter_context(tc.tile_pool(name="attn", bufs=1))
    Kt = c2.tile([128, NDT * S], BF16, tag="Kt")
    Qt = c2.tile([128, NDT * QB], BF16, tag="Qt")

    # right-side: router tensors
    re_ = rstk.enter_context(tc.tile_pool(name="router", bufs=1, side="right"))
    ctxT = re_.tile([128, NDT * QB], BF16, tag="ctxT")
    snn_sb = re_.tile([128, NQS * NN], F32, tag="snn")
    w_bf = re_.tile([128, NQS * NN], BF16, tag="w_bf")
    wT_sb = re_.tile([128, 2 * QB], BF16, tag="wT")
    wse1_sb = re_.tile([128, NDT * NN], BF16, tag="wse1")
    wse2_sb = re_.tile([128, NDT * NN], BF16, tag="wse2")
    bse_sb = re_.tile([1, NN], BF16, tag="bse")
    embw_sb = re_.tile([128, 2 * D], BF16, tag="embw")
    nc.sync.dma_start(wse1_sb[:], t["wse1"])
    nc.sync.dma_start(wse2_sb[:], t["wse2"])
    nc.sync.dma_start(bse_sb[:], t["bse"])
    for n in range(2):
        nc.sync.dma_start(embw_sb[:, D * n:D * (n + 1)],
                          t["embWnp"][128 * n:128 * (n + 1), :])

    kq_stack = ExitStack()
    wkq = kq_stack.enter_context(tc.tile_pool(name="wkq", bufs=3))
    PS = {}  # set to the attention stps pool; kq groups borrow its banks

    def kq_ps():
        return PS["stps"].tile([128, 1024], F32, tag="st", name="kqps")

    def q_group(j):
        wq_t = wkq.tile([128, NDT * 128], BF16, tag="wkq", name="wq_t")
        nc.sync.dma_start(wq_t[:], t["wqT"][128 * j:128 * (j + 1), :])
        ps = kq_ps()
        for i in range(NDT):
            nc.tensor.matmul(ps[:, 0:512], wq_t[:, 128 * i:128 * (i + 1)],
                             xho_bf[:, QB * i:QB * (i + 1)],
                             start=(i == 0), stop=(i == NDT - 1))
        nc.vector.tensor_scalar(Qt[:, QB * j:QB * (j + 1)], ps[:, 0:512],
                                cols["bq_c"][:, j:j + 1], None, A.add)

    wk_of = {}

    def k_group(j, c):
        if c == 0:
            wk_t = wkq.tile([128, NDT * 128], BF16, tag="wkq", name="wk_t")
            nc.sync.dma_start(wk_t[:], t["wkT"][128 * j:128 * (j + 1), :])
            wk_of[j] = wk_t
        wk_t = wk_of[j]
        ps = kq_ps()
        for i in range(NDT):
            nc.tensor.matmul(ps[:, 0:512], wk_t[:, 128 * i:128 * (i + 1)],
                             xhsl(i, c),
                             start=(i == 0), stop=(i == NDT - 1))
        nc.vector.tensor_scalar(Kt[:, S * j + 512 * c:S * j + 512 * (c + 1)],
                                ps[:, 0:512], cols["bk_c"][:, j:j + 1],
                                None, A.add)

    def snnx_group(qs):
        # xhat part of the factored router scores (no ctx dependency)
        ps = kq_ps()
        for i in range(NDT):
            nc.tensor.matmul(
                ps[:, 0:NN],
                xho_bf[:, QB * i + 128 * qs:QB * i + 128 * (qs + 1)],
                wse1_sb[:, NN * i:NN * (i + 1)],
                start=(i == 0), stop=False)
        nc.tensor.matmul(ps[:, 0:NN], ones_row[:], bse_sb[:],
                         start=False, stop=True)
        nc.vector.tensor_copy(snn_sb[:, NN * qs:NN * (qs + 1)], ps[:, 0:NN])

    # deferred K/Q (+router xhat-part) work queue: emitted between attention
    # steps so the PE always has dense full-array work while ACT runs exp.
    kq_work = []
    for j in range(NDT):
        kq_work.append((lambda j=j: q_group(j)))
        for c in range(4):
            kq_work.append((lambda j=j, c=c: k_group(j, c)))
    for qs in range(NQS):
        kq_work.append((lambda qs=qs: snnx_group(qs)))

    # ---- attention: head-pair row-tiled scores + [128,1024] exp tiles ----
    with tc.tile_pool(name="expp", bufs=3) as expp, \
         tc.tile_pool(name="denp", bufs=8) as denp, \
         tc.tile_pool(name="stps", bufs=2, space="PSUM") as stps, \
         tc.tile_pool(name="ctxps", bufs=4, space="PSUM") as ctxps:
        PS["stps"] = stps
        # head hp needs Q(j=hp) and K(j=hp, c=0..3) complete: pre-emit the
        # first two head-columns, interleave the rest.
        for _ in range(10):
            kq_work.pop(0)()
        steps = [(hp, tt) for hp in range(H // 2) for tt in range(NTT)]
        PIPE = 2
        st_of, eU_of, ctx_of = {}, {}, {}
        norm_q = []

        def emit_scores(k):
            hp, tt = steps[k]
            j = hp
            st = stps.tile([128, 1024], F32, tag="st", name="st")
            for u in range(2):  # head parity: disjoint PE row strips
                po = 64 * u
                nc.tensor.matmul(
                    st[:, 512 * u:512 * (u + 1)],
                    Kt[po:po + 64, S * j + 128 * tt:S * j + 128 * (tt + 1)],
                    Qt[po:po + 64, QB * j:QB * (j + 1)],
                    start=True, stop=True)
            eU = expp.tile([128, 1024], BF16, tag="eU", name="eU")
            nc.scalar.activation(eU[:], st[:], AF.Exp)
            eU_of[k] = eU

        def emit_av(k):
            hp, tt = steps[k]
            if tt == 0:
                ctx_of[hp] = [ctxps.tile([65, 512], F32, tag="ctx", name="ctx")
                              for _ in range(2)]
            eU = eU_of.pop(k)
            for u in range(2):
                h = 2 * hp + u
                nc.tensor.matmul(ctx_of[hp][u][:],
                                 Vx[:, VW * tt + 65 * h:VW * tt + 65 * (h + 1)],
                                 eU[:, 512 * u:512 * (u + 1)],
                                 start=(tt == 0), stop=(tt == NTT - 1))
            if tt == NTT - 1:
                j = hp
                for u in range(2):
                    po = 64 * u
                    ctx_ps = ctx_of[hp][u]
                    dst = ctxT[po:po + 64, QB * j:QB * (j + 1)]
                    # copy-only eviction frees the PSUM bank fast; the
                    # per-query 1/den normalization is split into small
                    # deferred pieces (one drained per step) so the slow
                    # DVE reciprocal never blocks K/Q evictions in the
                    # DVE FIFO behind it.
                    stg = denp.tile([64, 512], F32, tag="stg", name="stg")
                    nc.vector.tensor_copy(stg[:], ctx_ps[0:64, :])
                    den = denp.tile([1, 512], F32, tag="den", name="den")
                    nc.scalar.activation(den[:], ctx_ps[64:65, :], AF.Copy)
                    rbc = denp.tile([64, 512], F32, tag="rbc", name="rbc")

                    def mk(stg=stg, den=den, rbc=rbc, dst=dst):
                        yield lambda: nc.gpsimd.partition_broadcast(
                            rbc[:], den[:])
                        yield lambda: nc.vector.reciprocal(
                            rbc[0:32, :], rbc[0:32, :])
                        yield lambda: nc.vector.reciprocal(
                            rbc[32:64, :], rbc[32:64, :])
                        yield lambda: nc.vector.tensor_mul(
                            dst, stg[:], rbc[:])
                    norm_q.extend(mk())
                ctx_of.pop(hp)

        for k in range(len(steps)):
            emit_scores(k)
            if k >= PIPE:
                emit_av(k - PIPE)
            if norm_q:
                norm_q.pop(0)()
            # sprinkle K/Q groups: one per two attention steps keeps the
            # required K(j<=hp) comfortably ahead of the attention wave.
            if k % 2 == 0 and kq_work:
                kq_work.pop(0)()
        for k in range(len(steps) - PIPE, len(steps)):
            emit_av(k)
        while kq_work:
            kq_work.pop(0)()
        while norm_q:
            norm_q.pop(0)()

    kq_stack.close()
    if "dbg_Kt" in t:
        nc.sync.dma_start(t["dbg_Kt"], Kt[:])
        nc.sync.dma_start(t["dbg_Qt"], Qt[:])
        nc.sync.dma_start(t["dbg_ctxT"], ctxT[:])
        nc.sync.dma_start(t["dbg_xh"], xho_bf[:])
    attn_stack.close()  # free Kt/Vx/Qt/xhr
    ap2 = est.enter_context(tc.tile_pool(name="poolA2", bufs=1))
    xaugT = ap2.tile([128, NDT * QB], BF16, tag="xaugT")
    ident = ap2.tile([128, 128], BF16, tag="ident")
    make_identity(nc, ident[:])
    # FFN tensors + weight prefetch, overlapped with the router section.
    # Pool order (left stack, LIFO): hsb -> wdnp -> wupp; wupp closes right
    # after the up loop, wdnp/hsb live through the down phase.
    hsb = est.enter_context(tc.tile_pool(name="hsb", bufs=1))
    hT = hsb.tile([128, NFT * QB], BF16, tag="hT")
    xo2 = hsb.tile([128, NDT * QB], F32R, tag="xo2")
    wdn_stack = ExitStack()
    wdnp = wdn_stack.enter_context(tc.tile_pool(name="wdn", bufs=1))
    wdn_h = [wdnp.tile([128, 16 * D], BF16, tag="wdn", name=f"wdn{kb}")
             for kb in range(2)]
    wup_stack = ExitStack()
    wupp = wup_stack.enter_context(tc.tile_pool(name="wup", bufs=1))
    wup_all = wupp.tile([128, NFT * D], BF16, tag="wup_all")
    # all FFN weight prefetches ride the (now idle) sync queue in the
    # order they will be consumed
    for fb in range(2):
        nc.sync.dma_start(
            wup_all[:, 16 * D * fb:16 * D * (fb + 1)]
            .rearrange("p (f n) -> p f n", f=16),
            t["wupT"][2048 * fb:2048 * (fb + 1), :]
            .rearrange("(f p) n -> p f n", p=128))
    for kb in range(2):
        nc.sync.dma_start(
            wdn_h[kb][:].rearrange("p (k n) -> p k n", k=16),
            t["wdownT"][2048 * kb:2048 * (kb + 1), :]
            .rearrange("(k p) n -> p k n", p=128))
    nc.sync.dma_start(xo2[:], t["xT_own"])  # residual re-load

    # ============ P4: snn (factored comb@Ws.T@embT)  P5: topk  P6: info+xaug
    with tc.tile_pool(name="rtp", bufs=2) as rtp, \
         tc.tile_pool(name="aps", bufs=3, space="PSUM") as aps, \
         tc.tile_pool(name="snnps", bufs=2, space="PSUM") as snnps, \
         tc.tile_pool(name="trps", bufs=2, space="PSUM") as trps:
        if True:
            for qs in range(NQS):
                ps = snnps.tile([128, NN], F32, tag="snn", name="pssnn")
                for i in range(NDT):
                    nc.tensor.matmul(
                        ps[:],
                        ctxT[:, QB * i + 128 * qs:QB * i + 128 * (qs + 1)],
                        wse2_sb[:, NN * i:NN * (i + 1)],
                        start=(i == 0), stop=(i == NDT - 1))
                snn = snn_sb[:, NN * qs:NN * (qs + 1)]
                nc.vector.tensor_add(snn, ps[:], snn)

            for qs in range(NQS):
                snn = snn_sb[:, NN * qs:NN * (qs + 1)]
                t8 = rtp.tile([128, 8], F32, tag="t8", name="t8")
                nc.vector.max(t8[:], snn)
                nmx = rtp.tile([128, 1], F32, tag="nmx", name="nmx")
                nc.vector.tensor_scalar(nmx[:], t8[:, 0:1], -1.0, None, A.mult)
                snz = rtp.tile([128, NN], F32, tag="snz", name="snz")
                nc.vector.match_replace(out=snz[:], in_to_replace=t8[:],
                                        in_values=snn, imm_value=-1e30)
                e = rtp.tile([128, NN], F32, tag="e", name="e")
                nc.scalar.activation(e[:], snn, AF.Exp, bias=nmx[:])
                mask = rtp.tile([128, NN], F32, tag="mask", name="mask")
                nc.vector.tensor_tensor(mask[:], snn, snz[:], A.not_equal)
                wu = rtp.tile([128, NN], F32, tag="wu", name="wu")
                nc.vector.tensor_mul(wu[:], e[:], mask[:])
                ssum = rtp.tile([128, 1], F32, tag="ssum", name="ssum")
                nc.vector.tensor_reduce(ssum[:], wu[:], X, A.add)
                rcp = rtp.tile([128, 1], F32, tag="rcp", name="rcp")
                nc.vector.reciprocal(rcp[:], ssum[:])
                nc.vector.tensor_scalar(w_bf[:, NN * qs:NN * (qs + 1)], wu[:],
                                        rcp[:], None, A.mult)

            for qs in range(NQS):
                for n in range(2):
                    ps = trps.tile([128, 128], BF16, tag="tr", name="pstr")
                    nc.tensor.transpose(
                        ps[:],
                        w_bf[:, NN * qs + 128 * n:NN * qs + 128 * (n + 1)],
                        ident[:])
                    nc.vector.tensor_copy(
                        wT_sb[:, QB * n + 128 * qs:QB * n + 128 * (qs + 1)],
                        ps[:])

        # info @ Wnp.T folded host-side into embWnp = emb @ Wnp.T
        for j in range(NDT):
            ps = aps.tile([128, 512], F32, tag="a", name="psinfo")
            for n in range(2):
                nc.tensor.matmul(
                    ps[:],
                    embw_sb[:, D * n + 128 * j:D * n + 128 * (j + 1)],
                    wT_sb[:, QB * n:QB * (n + 1)],
                    start=(n == 0), stop=(n == 1))
            xa = xaugT[:, QB * j:QB * (j + 1)]
            # n2 + bnp = xhat*g2 + (b2 + bnp)
            nc.vector.tensor_scalar(xa, xho_bf[:, QB * j:QB * (j + 1)],
                                    cols["g2_c"][:, j:j + 1],
                                    cols["b2np_c"][:, j:j + 1], A.mult, A.add)
            nc.vector.tensor_add(xa, ps[:], xa)
        if "dbg_wbf" in t:
            nc.sync.dma_start(t["dbg_wbf"], w_bf[:])
            nc.sync.dma_start(t["dbg_wT"], wT_sb[:])
            nc.sync.dma_start(t["dbg_xaugT"], xaugT[:])

    rstk.close()  # free xhat + router tensors

    # ============ P10: FFN up (gelu) ============
    with tc.tile_pool(name="fps", bufs=2, space="PSUM") as fps:
        for f in range(NFT):
            ps = fps.tile([128, 512], F32, tag="f", name="psf")
            for i in range(NDT):
                nc.tensor.matmul(
                    ps[:], wup_all[:, D * f + 128 * i:D * f + 128 * (i + 1)],
                    xaugT[:, QB * i:QB * (i + 1)],
                    start=(i == 0), stop=(i == NDT - 1))
            nc.scalar.activation(hT[:, QB * f:QB * (f + 1)], ps[:],
                                 AF.Gelu, bias=bup_sb[:, f:f + 1])
        if "dbg_hT" in t:
            nc.sync.dma_start(t["dbg_hT"], hT[:])
    wup_stack.close()

    # ============ P11: FFN down + residual (j-outer, early output) ====
    with tc.tile_pool(name="ysb", bufs=1) as yp, \
         tc.tile_pool(name="ops", bufs=3, space="PSUM") as ops:
        yT_sb = yp.tile([128, NDT * QB], F32, tag="yT")
        for kb in range(2):
            for j in range(NDT):
                ps = ops.tile([128, 512], F32, tag="o", name="o")
                for k16 in range(16):
                    nc.tensor.matmul(
                        ps[:],
                        wdn_h[kb][:, D * k16 + 128 * j:D * k16 + 128 * (j + 1)],
                        hT[:, QB * (16 * kb + k16):QB * (16 * kb + k16 + 1)],
                        start=(k16 == 0), stop=(k16 == 15))
                yj = yT_sb[:, QB * j:QB * (j + 1)]
                if kb == 0:
                    nc.vector.scalar_tensor_tensor(
                        yj, ps[:], cols["bdown_c"][:, j:j + 1],
                        xo2[:, QB * j:QB * (j + 1)], op0=A.add, op1=A.add)
                else:
                    nc.vector.tensor_add(yj, ps[:], yj)
                    nc.sync.dma_start(t["yT"][128 * j:128 * (j + 1), :], yj)
    wdn_stack.close()

    est.close()


# ---------------- host side ----------------

def _pretile(wT, nblk):
    """[D, nblk*128] -> [nblk*128, D]: row j*128+p holds wT[:, 128j+p]-major
    layout, i.e. out[j*128+p, i*128+c] = wT[i*128+p, 128j+c]."""
    Din = wT.shape[0]
    return np.ascontiguousarray(
        wT.reshape(Din // 128, 128, nblk, 128).transpose(2, 1, 0, 3)
        .reshape(nblk * 128, Din))


def _prerow(xT):
    """[D, n] -> [128, (D/128)*n]: out[p, i*n+c] = xT[i*128+p, c]."""
    Din, n = xT.shape
    return np.ascontiguousarray(
        xT.reshape(Din // 128, 128, n).transpose(1, 0, 2).reshape(128, -1))


def prep_shared(inp):
    f = lambda a: np.ascontiguousarray(np.asarray(a, np.float32))
    bf = lambda a: np.ascontiguousarray(np.asarray(a, BF))
    cols8 = lambda v: np.ascontiguousarray(np.asarray(v, np.float32).reshape(NDT, 128).T)
    g1 = f(inp["g1"])
    b1 = f(inp["b1"])
    Wq, Wk, Wv = f(inp["Wq"]), f(inp["Wk"]), f(inp["Wv"])
    W = {}
    # LN1 affine folded into weights: W @ (xhat*g1 + b1) = (W*g1) @ xhat + W@b1
    W["wqT"] = bf(_pretile((Wq.T * g1[:, None]) * 0.125, NDT))
    W["bq_c"] = cols8((f(inp["bq"]) + Wq @ b1) * 0.125)
    W["wkT"] = bf(_pretile(Wk.T * g1[:, None], NDT))
    W["bk_c"] = cols8(f(inp["bk"]) + Wk @ b1)
    WvTg = Wv.T * g1[:, None]
    bv_eff = f(inp["bv"]) + Wv @ b1
    wv_ext = np.zeros((D, VW), np.float32)
    bv_ext = np.zeros((1, VW), np.float32)
    for h in range(H):
        wv_ext[:, 65 * h:65 * h + 64] = WvTg[:, 64 * h:64 * (h + 1)]
        bv_ext[0, 65 * h:65 * h + 64] = bv_eff[64 * h:64 * (h + 1)]
        bv_ext[0, 65 * h + 64] = 1.0
    W["wvT_ext"] = bf(_prerow(wv_ext))
    W["bv_ext"] = bf(bv_ext)
    Ws = f(inp["Ws"])
    Ws1, Ws2 = Ws[:, :D], Ws[:, D:]
    emb = f(inp["neuron_emb"])
    # router factored: scores = n1@Ws1.T@emb.T + ctx@Ws2.T@emb.T + bse
    W["wse1"] = bf(_prerow((Ws1.T * g1[:, None]) @ emb.T))
    W["wse2"] = bf(_prerow(np.ascontiguousarray(Ws2.T) @ emb.T))
    W["bse"] = bf(((f(inp["bs"]) + Ws1 @ b1) @ emb.T)[None, :])
    # info @ Wnp.T folded: embWnp = emb @ Wnp.T
    W["embWnp"] = bf(emb @ f(inp["Wnp"]).T)
    W["wupT"] = bf(_pretile(np.ascontiguousarray(f(inp["Wup"]).T), NFT))
    W["wdownT"] = bf(inp["Wdown"].T)
    W["b2np_c"] = cols8(f(inp["b2"]) + f(inp["bnp"]))
    W["g2_c"] = cols8(inp["g2"])
    W["bdown_c"] = cols8(inp["bdown"])
    W["bup_c"] = np.ascontiguousarray(f(inp["bup"]).reshape(NFT, 128).T)
    W["ones_col"] = np.ones((128, 1), np.float32)
    return W


_NC_CACHE = {}


def get_nc():
    if "nc" not in _NC_CACHE:
        _NC_CACHE["nc"] = build_program()
    return _NC_CACHE["nc"]


def make_in_maps(inputs):
    W = prep_shared(inputs)
    x = np.asarray(inputs["x"], np.float32)
    in_maps = []
    for c in range(8):
        b, qi = c // 4, c % 4
        q0 = qi * QB
        xT = np.ascontiguousarray(x[b].T)
        xTr = np.ascontiguousarray(np.concatenate([xT[:, q0:], xT[:, :q0]], axis=1))
        m = dict(W)
        m["x_all"] = _prerow(xTr)
        m["xT_own"] = _prerow(np.ascontiguousarray(xTr[:, 0:QB]))
        in_maps.append(m)
    return in_maps


def assemble_output(results, inputs):
    x = np.asarray(inputs["x"])
    y = np.zeros((B, S, D), np.float32)
    for c in range(8):
        b, qi = c // 4, c % 4
        y[b, qi * QB:(qi + 1) * QB, :] = results[c]["yT"].T
    return y.astype(x.dtype, copy=False)


def kernel(**inputs):
    nc = get_nc()
    in_maps = make_in_maps(inputs)
    res = run_bass_kernel_spmd(nc, in_maps, core_ids=list(range(8)))
    return assemble_output(res.results, inputs)
